# revision 42
# baseline (speedup 1.0000x reference)
"""Low-pass FFT filtering kernel for Trainium2 (8 NeuronCores).

Math: reference does, per (batch b, channel i), with X = x[b,:,:,i] (256x256):
    out_i = irfft(rfft(X, axis=0) * mask) + irfft(rfft(X, axis=1) * mask)
with mask keeping rfft modes 0..15 (ortho norm). That filter is an orthogonal
projection P = W @ W.T where W [256, 31] is the orthonormal basis
{1/sqrt(n), sqrt(2/n)cos(2pi k t/n), -sqrt(2/n)sin(2pi k t/n)}_{k=1..15}.
So  out_i = P @ X_i + X_i @ P = W @ (W.T @ X_i) + (X_i @ W) @ W.T.

Device schedule (per core = one batch, channel-major layouts), 8 chunks of
4 channels (1024 cols) each:
  phase 1:  C = W.T @ Xcm   [31, 1024]   (Xcm = x[b] as [m, (i, n)])
            D = W.T @ Xt    [31, 1024]   (Xt  = x[b] as [n, (i, m)])
            C and D share one 2-bank PSUM tile (rows 0:31 / 32:63); the W
            halves are grouped so consecutive matmuls reuse the stationary
            weights.
  phase 2:  out[m-tile, n'] per (i, j): single K=63 matmul with
     lhsT = [Wt_j ; 0 ; D_i,j]  (63 x 128),  rhs = [C_i ; 0 ; Wt] (63 x 256)
  which accumulates both terms in one PSUM pass.
The kernel is DMA-bound (~12.5 MB must move per core), so the schedule keeps
the DMA engines saturated: inputs stream on the SP HWDGE ring (xc) and the
Pool SWDGE ring (xt), constants load once into 4 rotating L/R buffer pairs,
and output DMAs trail the input issues on the SP ring so input prefetch has
strict FIFO priority on HBM. Act does the C/D PSUM->SBUF copies, DVE the
output casts, each as wide (1024-col) single instructions to amortize
per-instruction overhead.
Inputs/weights are fp16 on device (PE runs fp16 at full rate vs 4x-cost
fp32 LOW_HIGH mode); accumulation is fp32 in PSUM; the output is staged fp16
on device and upcast to fp32 on host (rel err ~7e-4 end to end).
Sharding: batch b -> core b (8 cores, no communication).
"""

import os
import sys
import types

import numpy as np

import concourse.bass as bass
import concourse.bacc as bacc
import concourse.tile as tile
from concourse import mybir
from concourse.bass_utils import run_bass_kernel_spmd

B, M, N, I = 8, 256, 256, 32
KMAX = 16           # modes kept: 0..15
R = 2 * KMAX - 1    # 31 real basis vectors
FREE = I * N        # 8192
NCHUNK = 8          # channel chunks
CW = FREE // NCHUNK     # 1024 cols = 4 channels per chunk
CH_PER_CHUNK = I // NCHUNK
F32 = mybir.dt.float32
F16 = mybir.dt.float16
NPDT = np.float16

LAST_RESULTS = None  # BassKernelResults of the most recent run (for test.py)


def _ensure_ntff_hook():
    """Provide antenv.axon_hooks if the image lacks it, so trace=True works."""
    try:
        from antenv.axon_hooks import get_axon_ntff_profile_hook  # noqa: F401
        return
    except ImportError:
        pass
    try:
        from trn_agent_boot.trn_boot import _ntff_profile_via_ctypes
        hook = _ntff_profile_via_ctypes("/opt/axon/libaxon_pjrt.so")
    except Exception:
        hook = None
    mod = types.ModuleType("antenv.axon_hooks")
    _state = {"hook": hook}
    mod.get_axon_ntff_profile_hook = lambda: _state["hook"]
    mod.set_axon_ntff_profile_hook = lambda h: _state.update(hook=h)
    sys.modules["antenv.axon_hooks"] = mod
    try:
        import antenv
        antenv.axon_hooks = mod
    except ImportError:
        pass


def _basis():
    t = np.arange(N)
    cols = [np.ones(N) / np.sqrt(N)]
    for k in range(1, KMAX):
        cols.append(np.sqrt(2.0 / N) * np.cos(2 * np.pi * k * t / N))
        cols.append(-np.sqrt(2.0 / N) * np.sin(2 * np.pi * k * t / N))
    return np.stack(cols, axis=1).astype(np.float32)  # [256, 31]


def _build_nc():
    nc = bacc.Bacc("TRN2", target_bir_lowering=False, debug=False,
                   enable_asserts=False, num_devices=8,
                   enable_partition_id=False)

    xc = nc.declare_dram_parameter("xc", [M, FREE], F16, isOutput=False)
    xt = nc.declare_dram_parameter("xt", [N, I * M], F16, isOutput=False)
    w2 = nc.declare_dram_parameter("w2", [128, 2 * R], F16, isOutput=False)
    wz = nc.declare_dram_parameter("wz", [R + 1, CW], F16, isOutput=False)
    zw = nc.declare_dram_parameter("zw", [R + 1, CW], F16, isOutput=False)
    out = nc.declare_dram_parameter("out", [M, FREE], F16, isOutput=True)

    with tile.TileContext(nc) as tc:
        with (
            tc.tile_pool(name="const", bufs=1) as constp,
            tc.tile_pool(name="xin", bufs=8) as xin,
            tc.tile_pool(name="oput", bufs=3) as outp,
            tc.tile_pool(name="pcd", bufs=2, space=bass.MemorySpace.PSUM) as pcdp,
            tc.tile_pool(name="p2", bufs=2, space=bass.MemorySpace.PSUM) as p2p,
        ):
            w2sb = constp.tile([128, 2 * R], F16)

            # 4 rotating L/R buffer pairs; the Wt rows load once (DMA into
            # pair 0, on-chip replication into pairs 1-3) and persist, only
            # the C/D rows are rewritten per chunk
            Lt = [constp.tile([63, CW], F16, tag=f"L{j}", name=f"L{j}")
                  for j in range(4)]
            Rt = [constp.tile([63, CW], F16, tag=f"R{j}", name=f"R{j}")
                  for j in range(4)]
            for j in range(4):
                nc.gpsimd.dma_start(out=Lt[j][0:32, :], in_=wz[:])
                nc.gpsimd.dma_start(out=Rt[j][31:63, :], in_=zw[:])

            W0 = w2sb[:, 0:R]
            W1 = w2sb[:, R:2 * R]

            # --- input DMA issues: xc (+ xt row-half 1) on the SP HWDGE
            # ring, all up-front; xt row-half 0 on the Act HWDGE ring, later
            # issues interleaved with the copies in the chunk loop. The tiny
            # w2 rides first on the Act ring so chunk 0's data leads the SP
            # queue. ---
            nc.scalar.dma_start(out=w2sb[:], in_=w2[:])
            xs, ts = [], []
            for g in range(NCHUNK):
                gsl = slice(g * CW, (g + 1) * CW)
                x = xin.tile([128, 2 * CW], F16, tag="x", name="x")
                t = xin.tile([128, 2 * CW], F16, tag="t", name="t")
                xs.append(x)
                ts.append(t)
                if g == 0:
                    # quarter-granularity first transfers: the first matmul
                    # needs only cols 0:512, and the DMA path ramps slowly
                    # in its first microseconds
                    for q in range(2):
                        qs = slice(q * 512, (q + 1) * 512)
                        nc.sync.dma_start(out=x[:, qs], in_=xc[0:128, qs])
                    nc.sync.dma_start(out=x[:, CW:2 * CW],
                                      in_=xc[128:256, gsl])
                else:
                    nc.sync.dma_start(out=x[:, 0:CW], in_=xc[0:128, gsl])
                    nc.sync.dma_start(out=x[:, CW:2 * CW],
                                      in_=xc[128:256, gsl])
                nc.sync.dma_start(out=t[:, CW:2 * CW], in_=xt[128:256, gsl])
                if g < 3:
                    nc.scalar.dma_start(out=t[:, 0:CW], in_=xt[0:128, gsl])

            outs_todo = []
            for g in range(NCHUNK):
                Lg = Lt[g % 4]
                Rg = Rt[g % 4]
                x = xs[g]
                t = ts[g]

                # remaining xt issues ride between the copies on Act
                if g + 3 < NCHUNK:
                    gn = slice((g + 3) * CW, (g + 4) * CW)
                    tn = ts[g + 3]
                    nc.scalar.dma_start(out=tn[:, 0:CW], in_=xt[0:128, gn])

                # phase 1: C fully first (its copy gates the next chunk's
                # phase 2), then D; each stream grouped by weight half for
                # stationary locality
                pcd = pcdp.tile([63, 2 * 512], F32, tag="cd")
                for rows, src, cpos in ((slice(0, R), x, 0),
                                        (slice(32, 63), t, 32)):
                    for h, Wh in ((0, W0), (1, W1)):
                        st = (h == 0)
                        sp = (h == 1)
                        for f in range(2):
                            fsl = slice(f * 512, (f + 1) * 512)
                            nc.tensor.matmul(
                                pcd[rows, fsl], Wh,
                                src[:, h * CW + f * 512:h * CW + (f + 1) * 512],
                                start=st, stop=sp,
                                tile_position=(0, cpos))
                    # wide single-instruction copy (Act) as soon as the
                    # stream completes; each spans both PSUM banks
                    if cpos == 0:
                        nc.scalar.copy(Rg[0:R, :], pcd[0:R, :])
                    else:
                        nc.scalar.copy(Lg[32:63, :], pcd[32:63, :])

                # phase 2 for the PREVIOUS chunk: keeps the PE fed while
                # this chunk's copies land
                if g > 0:
                    outs_todo.append(_phase2(nc, outp, p2p, Lt, Rt, g - 1,
                                             fine=False))
            outs_todo.append(_phase2(nc, outp, p2p, Lt, Rt, NCHUNK - 1,
                                     fine=True))

            # early-chunk output DMAs ride the Pool SWDGE ring (cheap
            # issues, separate queue) so they drain in parallel with input
            # prefetch without ever sitting ahead of an input in a FIFO.
            # Late chunks (>=5) ride the by-then-idle SP HWDGE ring so the
            # slow SWDGE queue is empty well before the epilogue barrier;
            # the last chunk additionally ships in quarters.
            for k, o, fine in outs_todo:
                ksl = slice(k * CW, (k + 1) * CW)
                if fine:
                    for j in range(2):
                        np_ = 2 if j == 0 else 4
                        h = CW // np_
                        for q in range(np_):
                            qsl = slice(k * CW + q * h, k * CW + (q + 1) * h)
                            osl = slice(j * CW + q * h, j * CW + (q + 1) * h)
                            nc.sync.dma_start(
                                out=out[j * 128:(j + 1) * 128, qsl],
                                in_=o[:, osl])
                elif k >= NCHUNK - 3:
                    nc.sync.dma_start(out=out[0:128, ksl], in_=o[:, 0:CW])
                    nc.sync.dma_start(out=out[128:256, ksl],
                                      in_=o[:, CW:2 * CW])
                else:
                    nc.gpsimd.dma_start(out=out[0:128, ksl], in_=o[:, 0:CW])
                    nc.gpsimd.dma_start(out=out[128:256, ksl],
                                        in_=o[:, CW:2 * CW])

    nc.finalize()
    return nc


def _phase2(nc, outp, p2p, Lt, Rt, k, fine):
    """Emit phase-2 matmuls + casts for chunk k. Returns (k, staging tile)."""
    Lg = Lt[k % 4]
    Rg = Rt[k % 4]
    o = outp.tile([128, 2 * CW], F16, tag="o", name="o")
    for j in range(2):
        p2 = p2p.tile([128, CW], F32, tag="p2", name="p2")
        for i in range(CH_PER_CHUNK):
            csl = slice(i * N, (i + 1) * N)
            jsl = slice(i * N + j * 128, i * N + (j + 1) * 128)
            nc.tensor.matmul(p2[:, csl], Lg[:, jsl], Rg[:, csl],
                             start=True, stop=True)
        if fine:
            # j0 in halves, j1 (the very last pieces) in quarters so the
            # final cast->DMA->semaphore chain is as short as possible
            np_ = 2 if j == 0 else 4
            h = CW // np_
            for q in range(np_):
                nc.vector.tensor_copy(
                    o[:, j * CW + q * h:j * CW + (q + 1) * h],
                    p2[:, q * h:(q + 1) * h])
        else:
            nc.vector.tensor_copy(o[:, j * CW:(j + 1) * CW], p2[:])
    return k, o, fine


_NC = None


def kernel(x: np.ndarray) -> np.ndarray:
    global _NC, LAST_RESULTS
    x = np.asarray(x)
    assert x.shape == (B, M, N, I), x.shape

    W = _basis().astype(NPDT)          # [256, 31]
    Wt = W.T.copy()                    # [31, 256]
    w2_np = np.concatenate([W[0:128, :], W[128:256, :]], axis=1)  # [128, 62]
    wtile = np.tile(Wt, (1, CH_PER_CHUNK))                        # [31, 1024]
    wz_np = np.concatenate([wtile, np.zeros((1, CW), NPDT)], axis=0)
    zw_np = np.concatenate([np.zeros((1, CW), NPDT), wtile], axis=0)

    if _NC is None:
        _NC = _build_nc()

    xq = np.asarray(x, dtype=NPDT)
    in_maps = []
    for b in range(B):
        xcm = np.ascontiguousarray(xq[b].transpose(0, 2, 1)).reshape(M, FREE)
        xtm = np.ascontiguousarray(xq[b].transpose(1, 2, 0)).reshape(N, I * M)
        in_maps.append({
            "xc": xcm, "xt": xtm,
            "w2": w2_np, "wz": wz_np, "zw": zw_np,
        })

    trace = bool(int(os.environ.get("KERNEL_TRACE", "0")))
    if trace:
        _ensure_ntff_hook()
    last_err = None
    for attempt in range(3):
        try:
            LAST_RESULTS = run_bass_kernel_spmd(_NC, in_maps, list(range(B)),
                                                trace=trace and attempt == 0)
            break
        except Exception as e:  # rare transient NRT_EXEC_UNIT_UNRECOVERABLE
            last_err = e
            import time as _time
            _time.sleep(2.0)
            try:
                import jax
                jax.clear_caches()
                jax.extend.backend.clear_backends()
            except Exception:
                pass
    else:
        raise last_err

    out = np.empty((B, M, N, I), np.float32)
    for b in range(B):
        dev = LAST_RESULTS.results[b]["out"].astype(np.float32).reshape(M, I, N)
        out[b] = dev.transpose(0, 2, 1)
    return out
